# revision 61
# baseline (speedup 1.0000x reference)
"""Distance-based attention (nn_Attention_67989332296336) on 8 TRN2 NeuronCores.

Math per batch element b (S=1024, E=H=A=256):
    d2[t,j]  = |x_t|^2 + |x_j|^2 - 2 x_t.x_j
    dist     = sqrt(max(d2,0)+eps)
    scores   = w_sim*dist + b_sim     (b_sim cancels in softmax)
    A        = softmax_j(scores)
    G        = A @ h
    Z        = tanh([G, h] @ W_g^T + b_g)

Sharding: batch dim B=32 split over 8 cores (4 per core), weights replicated.

Per-core design (two software-pipelined stages):
  - Symmetry: dist/P are symmetric per batch, so gram, sqrt and exp run
    only on the lower block-triangle (36/64 tiles), stored compacted as
    [128, 4608] per batch.  PV lhsT tiles for k >= i come straight out
    of the compact store (tile (k,i) is a column-slice of row-block k);
    the 28 missing k < i tiles are 128x128 PE transposes (bf16)
    copied out of one PSUM bank by ScalarE (i<=3) / DVE (i>=4).
  - Gram runs in fp8 DoubleRow (both 128-row k-tiles in one MM, 2x PE
    throughput at FD=512): xT is downcast bf16->fp8 in the PSUM->SBUF
    transpose copies (split ScalarE/DVE).  MARGIN=16 keeps the d2
    diagonal positive against the bf16-|x|^2 vs fp8-gram mismatch
    (TRN2 sqrt(neg) = NaN); softmax distortion <0.2%.
  - d2 assembled in PSUM: -0.5|x_j|^2 folded in via a rank-8
    block-diagonal aug matmul, |x_t|^2 + MARGIN applied as the sqrt
    activation's per-partition bias (scale=-2).
  - Stage-1 row interleave: each gram row is followed by one hW m-tile
    (lagged 2 rows; hT transpose groups at rows 1/5) and a 2-tile
    transpose group of the next batch's xT, so the PE stream stays
    dense against the ScalarE sqrt pacing and the HAM clock gate never
    re-throttles mid-stage.  b_g is folded into the hw PSUM copy
    (alternating full-DVE STT / split ACT-copy + DVE-STT per m).
  - ScalarE runs exactly two table sets: Sqrt (+copies) in stage 1,
    Exp/Copy/Tanh in stage 2, pinned by same-engine chain deps.  exp is
    one [128,4608] instruction per batch, pipelined one batch ahead of
    PV; a 32-MM dependency-free heater bridges the exp(0) gate at the
    stage boundary so PV(0) enters warm.
  - Gate folded into PV: Z = tanh((P@hW1)/den + h@W2' + bg), with
    denominators from a ones-column in the PV rhs.
  - Host-side prep (pure layout, no compute): x/h/out use a p-major row
    permutation ([b,p,i,e]) so DMA descriptors are 2-4KB instead of
    512B (the input stream is packet-bound); identb/w12t/bgz constants
    are host-built and loaded via the scalar HWDGE queue, keeping the
    GpSimd descgen queue for x/h and the sync queue for the
    latency-critical bd diag writes.
  - HAM discipline: ~4us warmup burst up front; the interleaved stage-1
    stream plus the boundary heater keep the PE at 8/8 after the ramp.
"""

import sys

import numpy as np

if "/opt/trn_rl_repo" not in sys.path:
    sys.path.append("/opt/trn_rl_repo")

import concourse.bacc as bacc
import concourse.bass as bass
import concourse.mybir as mybir
import concourse.tile as tile
from concourse.bass import ts
from concourse.bass_utils import run_bass_kernel_spmd

import ml_dtypes

BF16_NP = ml_dtypes.bfloat16
try:
    FP8_NP = ml_dtypes.float8_e4m3
except AttributeError:
    FP8_NP = ml_dtypes.float8_e4m3fn

F32 = mybir.dt.float32
BF16 = mybir.dt.bfloat16
FP8 = mybir.dt.float8e4
DR = mybir.MatmulPerfMode.DoubleRow
AF = mybir.ActivationFunctionType
OP = mybir.AluOpType

S = 1024
B = 32
NCORES = 8
BS = B // NCORES  # batches per core
E = 256
H = 256
A = 256
NT = S // 128  # 8 t-tiles
# replaces max(d2,0)+eps; absorbs rounding so the d2 diagonal (where the
# bf16 |x|^2 bias meets the fp8 gram, mismatch RMS ~2) can never go
# negative into Sqrt (TRN2 sqrt(neg) = NaN).  Distorts softmax by <0.2%.
MARGIN = 16.0

# lower-triangle compact layout: row-block i holds cols j in [0, 128*(i+1))
NI = [128 * (i + 1) for i in range(NT)]
OFF = [64 * i * (i + 1) for i in range(NT)]  # sum of NI[:i]
TRI = OFF[NT - 1] + NI[NT - 1]  # 4608
LOFF = [64 * i * (i - 1) for i in range(NT)]  # p_lo group offset: 128*(0+1+..+i-1)
NLO = 28 * 128  # 3584


def _chain(prev, cur, reason):
    """Pin same-engine execution order (table-set discipline)."""
    if prev is not None:
        tile.add_dep_helper(cur.ins, prev.ins, sync=False, reason=reason)
    return cur


def build_graph():
    nc = bacc.Bacc("TRN2", target_bir_lowering=False, debug=False)

    # x/h arrive p-major ([b, p, i, e], a pure host-side row permutation of
    # the shard) so each DMA descriptor covers a full partition chunk
    # (2-4KB) instead of 512B lines -- the input stream is packet-bound.
    x_ext = nc.declare_dram_parameter("x", [BS, 128, NT, E], F32, isOutput=False)
    h_ext = nc.declare_dram_parameter("h", [BS, 128, NT, H], F32, isOutput=False)
    w_ext = nc.declare_dram_parameter("w_sim", [1, 1], F32, isOutput=False)
    # host-prepared constants (weight layout prep done on CPU): keeps the
    # GpSimd descriptor-gen queue free for the x/h input stream at startup
    idb_ext = nc.declare_dram_parameter("identb", [128, 128], BF16, isOutput=False)
    w12t_ext = nc.declare_dram_parameter("w12t", [128, 2, 2 * H], BF16, isOutput=False)
    bgz_ext = nc.declare_dram_parameter("bgz", [128, 2, A], BF16, isOutput=False)
    # output is p-major too ([b, p, i, a]); host un-permutes after gather
    out_ext = nc.declare_dram_parameter("out", [BS, 128, NT, A], F32, isOutput=True)

    with tile.TileContext(nc) as tc:
        with (
            tc.tile_pool(name="consts", bufs=1) as consts,
            tc.tile_pool(name="nat", bufs=BS) as natp,
            tc.tile_pool(name="xt", bufs=2) as xtp,
            tc.tile_pool(name="small", bufs=2) as smallp,
            tc.tile_pool(name="dcmp", bufs=BS) as dcmpp,
            tc.tile_pool(name="pcmp", bufs=2) as pcmpp,
            tc.tile_pool(name="plo", bufs=2) as plop,
            tc.tile_pool(name="hw", bufs=BS) as hwp,
            tc.tile_pool(name="zt", bufs=2) as ztp,
            tc.tile_pool(name="ps_tr", bufs=2, space="PSUM") as pstr,
        ):
            big_ctx = tc.tile_pool(name="ps_big", bufs=2, space="PSUM")
            psbig = big_ctx.__enter__()
            # ---- PE HAM warm-up: dense burst while input DMAs fly ----
            warm_in = consts.tile([128, 128], BF16)
            nc.vector.memset(warm_in, 1.0)
            warm_ps = psbig.tile([128, 512], F32, tag="big")
            for _ in range(44):
                nc.tensor.matmul(
                    warm_ps[:, 0:128], warm_in[:], warm_in[:], start=True, stop=True
                )

            # ---- constant DMAs on the scalar HWDGE queue: keeps the sync
            # queue free for the latency-critical bd diag writes ----
            identb = consts.tile([128, 128], BF16)
            nc.scalar.dma_start(out=identb, in_=idb_ext[:])
            w12t = consts.tile([128, 2, 2 * H], BF16)
            nc.scalar.dma_start(out=w12t, in_=w12t_ext[:])
            bgz = consts.tile([128, 2, A], BF16)
            nc.scalar.dma_start(out=bgz, in_=bgz_ext[:])
            w_col = consts.tile([128, 1], F32)
            nc.scalar.dma_start(out=w_col, in_=w_ext[:].partition_broadcast(128))

            # ---- input DMAs (SWDGE casting, in consumption order) ----
            # All buffers are fresh (bufs=BS) so every trigger is emitted
            # up-front with no WAR gating; descgen on the Q7 is the only
            # serializer (~1.2us each).
            xnat_list, hnat_list = [], []
            for b in range(BS):
                xnat = natp.tile([128, NT, E], BF16, tag="xnat", name=f"xnat{b}")
                xnat_list.append(xnat)
                hnat = natp.tile([128, NT, H], BF16, tag="hnat", name=f"hnat{b}")
                hnat_list.append(hnat)

            # batch 0's x arrives in finer chunks so the first transposes can
            # start right as warmup ends
            XCHUNKS = {0: ((0, 2), (2, 2), (4, 4))}
            XDEFAULT = ((0, 4), (4, 4))

            def emit_in_dma(*bs):
                for b in bs:
                    for i0, gn in XCHUNKS.get(b, XDEFAULT):
                        nc.gpsimd.dma_start(
                            out=xnat_list[b][:, i0 : i0 + gn, :],
                            in_=x_ext[b, :, i0 : i0 + gn, :],
                        )
                    nc.gpsimd.dma_start(
                        out=hnat_list[b],
                        in_=h_ext[b, :, :, :],
                    )

            emit_in_dma(0, 1, 2, 3)

            # ---------------- constants ----------------
            ones_stage = consts.tile([8, 128], F32)
            nc.vector.memset(ones_stage, 1.0)
            ones8 = consts.tile([8, 128], BF16)
            nc.vector.tensor_copy(ones8, ones_stage)

            # force the Sqrt table set to load once, up front
            dummy = consts.tile([128, 1], F32)
            sc_prev = nc.scalar.activation(out=dummy, in_=w_col, func=AF.Sqrt)
            sc_box0 = [sc_prev]  # stage-1 ScalarE same-engine order chain

            # -------- per-batch state --------
            d_cmp = [dcmpp.tile([128, TRI], BF16, tag="d", name=f"dcmp{b}") for b in range(BS)]
            hw_l = [hwp.tile([128, NT, 520], BF16, tag="hw", name=f"hw{b}") for b in range(BS)]

            def transpose_nat(nat, dstT, groups=XDEFAULT):
                """nat [128, NT, 256] -> dstT [128, 2, S] (k-major).

                Transposes run in bf16; if dstT is fp8 the PSUM->SBUF copy
                does the downcast, alternating ScalarE/DVE per group to
                split the cast cost.
                """
                fp8 = dstT.dtype == FP8
                for gi, (i0, gn) in enumerate(groups):
                    psT = pstr.tile([128, 1024], BF16, tag="tr")
                    for i2 in range(gn):
                        i = i0 + i2
                        for k2 in range(2):
                            nc.tensor.transpose(
                                psT[:, i2 * 256 + k2 * 128 : i2 * 256 + (k2 + 1) * 128],
                                nat[:, i, ts(k2, 128)],
                                identb[:],
                            )
                    dst = dstT[:, :, 128 * i0 : 128 * (i0 + gn)].rearrange(
                        "p k (i f) -> p i k f", i=gn
                    )
                    src = psT[:, 0 : 256 * gn].rearrange(
                        "p (i k f) -> p i k f", i=gn, k=2
                    )
                    if fp8 and (i0 // 2) % 2 == 0:
                        ci = nc.scalar.copy(dst, src)
                        sc_box0[0] = _chain(sc_box0[0], ci, "act-order")
                    else:
                        nc.vector.tensor_copy(dst, src)

            def emit_hw_row(b, hT, m):
                """One m-tile of hW = h @ [W1|W2]^T (+ bg, + ones column)."""
                hw = hw_l[b]
                ps = psbig.tile([128, 512], F32, tag="big")
                nc.tensor.matmul(
                    ps[:], hT[:, 0, ts(m, 128)], w12t[:, 0, :],
                    start=True, stop=False,
                )
                nc.tensor.matmul(
                    ps[:], hT[:, 1, ts(m, 128)], w12t[:, 1, :],
                    start=False, stop=True,
                )
                hwm = hw[:, m, :]
                if m % 2 == 0:
                    dst = bass.AP(
                        tensor=hwm.tensor,
                        offset=hwm.offset,
                        ap=[hwm.ap[0], [257, 2], [1, 256]],
                    )
                    # hw = ps + [0 | bg]: bg folded here instead of a PE matmul
                    nc.vector.scalar_tensor_tensor(
                        out=dst,
                        in0=ps[:].rearrange("p (u f) -> p u f", u=2),
                        scalar=1.0,
                        in1=bgz[:],
                        op0=OP.mult,
                        op1=OP.add,
                    )
                else:
                    # split: ScalarE takes the G half, DVE adds bg to W2 half
                    ci = nc.scalar.copy(hw[:, m, 0:256], ps[:, 0:256])
                    sc_box0[0] = _chain(sc_box0[0], ci, "act-order")
                    nc.vector.scalar_tensor_tensor(
                        out=hw[:, m, 257:513],
                        in0=ps[:, 256:512],
                        scalar=1.0,
                        in1=bgz[:, 1, :],
                        op0=OP.mult,
                        op1=OP.add,
                    )

            # ================= stage 1: x->dist triangle (+ hW) =================
            # hW block for batch b+1 is emitted after gram(b): each is a
            # >=3.4us dense PE burst that re-warms the clock gate, and
            # do_hw(0) runs right after warmup while x0 is still landing.
            stage1 = tc.tile_pool(name="ps_d2", bufs=2, space="PSUM")
            psd2 = stage1.__enter__()
            xT_l = {}

            def alloc_xT(b):
                xT_l[b] = xtp.tile([128, 2, S], FP8, tag="xT", name=f"xT{b}")
                return xT_l[b]

            alloc_xT(0)
            # batch 0 startup: only tiles 0-3 transpose before the first sq
            # section, so the sq->sq4->bd-diag chain (which gates the first
            # gram rows) launches as soon as x0's first chunks land
            transpose_nat(xnat_list[0], xT_l[0], groups=((0, 2), (2, 2)))
            sq_state = {}

            def emit_sq_section(b):
                """|x_t|^2 (DVE) -> bias half + blockdiag half, per x-chunk g.

                Called one batch ahead (inside batch b-1's row loop) so the
                ~2us bd diag-DMA roundtrip is off the batch boundary."""
                xnat = xnat_list[b]
                bias_h = []
                bd_h = []
                for g in range(2):
                    if b == 0 and g == 1:
                        transpose_nat(xnat_list[0], xT_l[0], groups=((4, 4),))
                    sqm = smallp.tile([128, 4], F32, tag=f"sqm{g}", name=f"sqm{g}")
                    for u in range(4):
                        i = 4 * g + u
                        scr = smallp.tile([128, E], BF16, tag="scr")
                        nc.vector.scalar_tensor_tensor(
                            out=scr,
                            in0=xnat[:, i, :],
                            scalar=1.0,
                            in1=xnat[:, i, :],
                            op0=OP.mult,
                            op1=OP.mult,
                            accum_out=sqm[:, u : u + 1],
                        )
                    biasg = smallp.tile([128, 4], F32, tag=f"bias{g}", name=f"bias{g}")
                    bias_h.append(biasg)
                    nc.vector.tensor_scalar_add(out=biasg, in0=sqm, scalar1=MARGIN)
                    sqmb = smallp.tile([128, 4], BF16, tag=f"sqmb{g}", name=f"sqmb{g}")
                    nc.vector.tensor_scalar_mul(sqmb[:], sqm[:], -0.5)
                    sq4 = pstr.tile([4, 128], BF16, tag="tr", name="sq4")
                    nc.tensor.transpose(sq4[:], sqmb[:], identb[:])
                    sq4sb = smallp.tile([4, 128], BF16, tag=f"sq4sb{g}", name=f"sq4sb{g}")
                    nc.vector.tensor_copy(sq4sb[:], sq4[:])
                    bdg = smallp.tile([8, 512], BF16, tag=f"bd{g}", name=f"bd{g}")
                    bd_h.append(bdg)
                    if b < 2:
                        nc.vector.memset(bdg, 0.0)
                    bd = bdg[:]
                    diag_view = bass.AP(
                        tensor=bd.tensor, offset=bd.offset, ap=[[512 + 128, 4], [1, 128]]
                    )
                    nc.sync.dma_start(out=diag_view, in_=sq4sb[:])
                sq_state[b] = (bias_h, bd_h)

            emit_sq_section(0)
            for b in range(BS):
                xnat = xnat_list[b]
                xT = xT_l[b]
                nxT = alloc_xT(b + 1) if b + 1 < BS else None
                hT = xtp.tile([128, 2, S], BF16, tag="hT", name=f"hT{b}")
                nc.vector.memset(hw_l[b][:, :, 256:257], 1.0)
                bias_h, bd_h = sq_state[b]

                if b == 0:
                    # fill the first bd-diag DMA wait with dependency-free
                    # matmuls so the clock gate never sees a >3.4us idle on
                    # the ramp (cold batches 0-1 otherwise cost ~10us)
                    hp0 = psbig.tile([128, 512], F32, tag="big", name="ramp_heat")
                    for _ in range(48):
                        nc.tensor.matmul(
                            hp0[:, 0:128], warm_in[:], warm_in[:],
                            start=True, stop=True,
                        )

                # gram rows (lower triangle) -> d2 psum -> sqrt -> compact dist
                for i in range(NT):
                    ni = NI[i]
                    d2 = psd2.tile([128, 1024], F32, tag="d2")
                    nchunk = (ni + 511) // 512
                    for c in range(nchunk):
                        j0 = 512 * c
                        j1 = min(ni, j0 + 512)
                        # fp8 DoubleRow: both 128-row k-tiles in one matmul
                        nc.tensor.matmul(
                            d2[:, j0:j1],
                            xT[:, :, ts(i, 128)],
                            xT[:, :, j0:j1],
                            start=True,
                            stop=False,
                            perf_mode=DR,
                        )
                        nc.tensor.matmul(
                            d2[:, j0:j1],
                            ones8[:],
                            bd_h[c][:, 0 : j1 - j0],
                            start=False,
                            stop=True,
                        )
                    si = nc.scalar.activation(
                        out=d_cmp[b][:, OFF[i] : OFF[i] + ni],
                        in_=d2[:, 0:ni],
                        func=AF.Sqrt,
                        bias=bias_h[i // 4][:, i % 4 : i % 4 + 1],
                        scale=-2.0,
                    )
                    sc_box0[0] = _chain(sc_box0[0], si, "act-order")
                    # interleave hW m-tiles (lagged by 2 so the hT transposes
                    # emitted at rows 1/5 never block batch 0's gram on the
                    # late-landing h DMA): the PE stream stays dense against
                    # the sqrt/psum pacing (no HAM re-throttle); next batch's
                    # xT transposes fill rows 2-5
                    if i == 1:
                        transpose_nat(hnat_list[b], hT, groups=((0, 4),))
                    elif i == 5:
                        transpose_nat(hnat_list[b], hT, groups=((4, 4),))
                    if i >= 2:
                        emit_hw_row(b, hT, i - 2)
                    if nxT is not None and 2 <= i < 6:
                        transpose_nat(
                            xnat_list[b + 1], nxT, groups=((2 * (i - 2), 2),)
                        )
                    if i == 6 and b + 1 < BS:
                        emit_sq_section(b + 1)
                emit_hw_row(b, hT, 6)
                emit_hw_row(b, hT, 7)
            stage1.__exit__(None, None, None)
            big_ctx.__exit__(None, None, None)
            stage2 = tc.tile_pool(name="ps_pv", bufs=6, space="PSUM")
            pspv = stage2.__enter__()

            # ================= stage 2: exp, PV, gate =================
            # dependency-free burst bridges the exp(0) gate (~4.2us) at the
            # stage boundary so the PE clock stays at 8/8 into PV(0)
            bh = pspv.tile([128, 512], F32, tag="pv", name="boundary_heat")
            for _ in range(64):
                nc.tensor.matmul(
                    bh[:, 0:128], warm_in[:], warm_in[:], start=True, stop=True
                )
            sc_box = sc_box0
            p_cmp_l = {}

            def emit_exp(b):
                if b in p_cmp_l:
                    return
                pc = pcmpp.tile([128, TRI], BF16, tag="p", name=f"pcmp{b}")
                ei = nc.scalar.activation(
                    out=pc, in_=d_cmp[b], func=AF.Exp, scale=w_col[:, 0:1]
                )
                sc_box[0] = _chain(sc_box[0], ei, "act-order")
                p_cmp_l[b] = pc

            p_lo_l = {}

            def get_plo(b):
                if b not in p_lo_l:
                    p_lo_l[b] = plop.tile([128, NLO], BF16, tag="plo", name=f"plo{b}")
                return p_lo_l[b]

            emitted_tr = set()

            def emit_transp(b, i):
                # transpose the missing lhsT tiles (k < i) for PV_i of batch b
                if (b, i) in emitted_tr:
                    return
                emitted_tr.add((b, i))
                p_cmp = p_cmp_l[b]
                p_lo = get_plo(b)
                psT = pstr.tile([128, 1024], BF16, tag="tr")
                for k in range(i):
                    nc.tensor.transpose(
                        psT[:, ts(k, 128)],
                        p_cmp[:, OFF[i] + 128 * k : OFF[i] + 128 * (k + 1)],
                        identb[:],
                    )
                dst = p_lo[:, LOFF[i] : LOFF[i] + 128 * i]
                if i <= 3:
                    ci = nc.scalar.copy(dst, psT[:, 0 : 128 * i])
                    sc_box[0] = _chain(sc_box[0], ci, "act-order")
                else:
                    nc.vector.tensor_copy(dst, psT[:, 0 : 128 * i])

            emit_exp(0)
            for b in range(BS):
                p_cmp = p_cmp_l[b]
                p_lo = get_plo(b)
                hw = hw_l[b]

                zs = None
                for i in range(NT):
                    if i + 1 < NT:
                        emit_transp(b, i + 1)
                    elif b + 1 < BS:
                        # lookahead: next batch's first transpose group + copy,
                        # so its p_lo copy lands before this batch's last tanh
                        # in the ScalarE chain (avoids the merged-wait stall).
                        emit_transp(b + 1, 1)
                    if i == 3 and b + 1 < BS:
                        emit_exp(b + 1)
                    if i % 4 == 0:
                        zs = ztp.tile([128, 4, A], F32, tag="zs", bufs=3)
                    pv = pspv.tile([128, 512], F32, tag="pv")
                    for k in range(NT):
                        if k >= i:
                            lhsT = p_cmp[:, OFF[k] + 128 * i : OFF[k] + 128 * (i + 1)]
                        else:
                            lhsT = p_lo[:, LOFF[i] + 128 * k : LOFF[i] + 128 * (k + 1)]
                        nc.tensor.matmul(
                            pv[:, 0 : A + 1],
                            lhsT,
                            hw[:, k, 0 : A + 1],
                            start=(k == 0),
                            stop=(k == NT - 1),
                        )
                    rp_i = smallp.tile([128, 1], F32, tag="rp_i")
                    nc.vector.reciprocal(rp_i[:], pv[:, A : A + 1])
                    nc.vector.scalar_tensor_tensor(
                        out=zs[:, i % 4, :],
                        in0=pv[:, 0:A],
                        scalar=rp_i[:, 0:1],
                        in1=hw[:, i, 257 : 257 + A],
                        op0=OP.mult,
                        op1=OP.add,
                    )
                    last_half = b == BS - 1 and i >= 4
                    if last_half and i in (5, 7):
                        # final batch: drain the output in 2-row-tile pieces,
                        # split across the SWDGE and HWDGE queues so the last
                        # piece moves at ~2x queue bandwidth
                        q = (i - 5) // 2
                        if q == 0:
                            zo = ztp.tile([128, 4, A], F32, tag="zo")
                        ti = nc.scalar.activation(
                            out=zo[:, 2 * q : 2 * q + 2, :].rearrange("p a b -> p (a b)"),
                            in_=zs[:, 2 * q : 2 * q + 2, :].rearrange("p a b -> p (a b)"),
                            func=AF.Tanh,
                        )
                        sc_box[0] = _chain(sc_box[0], ti, "act-order")
                        i0 = 4 + 2 * q
                        nc.sync.dma_start(
                            out=out_ext[b, :, i0, :],
                            in_=zo[:, 2 * q, :],
                        )
                        nc.gpsimd.dma_start(
                            out=out_ext[b, :, i0 + 1, :],
                            in_=zo[:, 2 * q + 1, :],
                        )
                    elif i % 4 == 3 and not last_half:
                        g2 = i // 4
                        zo = ztp.tile([128, 4, A], F32, tag="zo")
                        ti = nc.scalar.activation(
                            out=zo[:].rearrange("p a b -> p (a b)"),
                            in_=zs[:].rearrange("p a b -> p (a b)"),
                            func=AF.Tanh,
                        )
                        sc_box[0] = _chain(sc_box[0], ti, "act-order")
                        for q in range(2):
                            i0 = 4 * g2 + 2 * q
                            eng = nc.sync if q == 0 else nc.gpsimd
                            eng.dma_start(
                                out=out_ext[b, :, i0 : i0 + 2, :],
                                in_=zo[:, 2 * q : 2 * q + 2, :],
                            )
                if b < BS - 1:
                    hp2 = pspv.tile([128, 512], F32, tag="pv", name="heat2")
                    for _ in range(12):
                        nc.tensor.matmul(
                            hp2[:, 0:128], warm_in[:], warm_in[:], start=True, stop=True
                        )
            stage2.__exit__(None, None, None)

    nc.compile()
    return nc


_CACHED = {}


def _get_graph():
    if "nc" not in _CACHED:
        _CACHED["nc"] = build_graph()
    return _CACHED["nc"]


def _run(inputs, trace=False, **kw):
    nc = _get_graph()
    x = np.asarray(inputs["x"], dtype=np.float32)
    h = np.asarray(inputs["h"], dtype=np.float32)
    w_sim = np.asarray(inputs["w_sim"], dtype=np.float32).reshape(1, 1)
    W_g = np.ascontiguousarray(np.asarray(inputs["W_g"], dtype=np.float32))
    b_g = np.asarray(inputs["b_g"], dtype=np.float32).reshape(1, A)
    # host-side constant prep (weight layout): identity, transposed W_g,
    # and the [0 | bg] broadcast tile
    identb = np.eye(128, dtype=BF16_NP)
    # w12t[r, k2, w*256 + a] = W_g[a, w*256 + k2*128 + r]
    w12t = np.ascontiguousarray(
        W_g.reshape(A, 2, 2, 128).transpose(3, 2, 1, 0).reshape(128, 2, 2 * H)
    ).astype(BF16_NP)
    bgz = np.zeros((128, 2, A), dtype=np.float32)
    bgz[:, 1, :] = b_g
    bgz = bgz.astype(BF16_NP)
    # p-major row permutation: [S, BS, E] -> [BS, 128, NT, E] with
    # row (i*128+p, b) -> [b, p, i]; gives 2-4KB DMA descriptors
    def pmajor(t):
        return np.ascontiguousarray(
            t.reshape(NT, 128, t.shape[1], t.shape[2]).transpose(2, 1, 0, 3)
        )

    in_maps = []
    for c in range(NCORES):
        in_maps.append(
            {
                "x": pmajor(x[:, c * BS : (c + 1) * BS, :]),
                "h": pmajor(h[:, c * BS : (c + 1) * BS, :]),
                "w_sim": w_sim,
                "identb": identb,
                "w12t": w12t,
                "bgz": bgz,
            }
        )
    res = run_bass_kernel_spmd(nc, in_maps, list(range(NCORES)), trace=trace, **kw)
    # out arrives p-major [BS, 128, NT, A] per core; un-permute to [S, BS, A]
    outs = []
    for c in range(NCORES):
        o = res.results[c]["out"]  # [BS, 128, NT, A]
        outs.append(o.transpose(2, 1, 0, 3).reshape(S, BS, A))
    out = np.concatenate(outs, axis=1)
    return out, res


def kernel(**inputs):
    out, _ = _run(inputs, trace=False)
    return out


if __name__ == "__main__":
    rng = np.random.default_rng(0)
    ins = {
        "x": rng.standard_normal((S, B, E), dtype=np.float32),
        "h": rng.standard_normal((S, B, H), dtype=np.float32),
        "w_sim": np.array([0.03], dtype=np.float32),
        "b_sim": np.array([0.01], dtype=np.float32),
        "W_g": (rng.standard_normal((A, 2 * H)) * 0.05).astype(np.float32),
        "b_g": np.zeros(A, dtype=np.float32),
    }
    out = kernel(**ins)
    print("out", out.shape, out.dtype, np.abs(out).mean())

